# revision 18
# baseline (speedup 1.0000x reference)
"""Trainium2 kernel for nn_NNLoss (brute-force NN + margin loss).

loss = mean(relu(margin - max(min_j |q_i - m_j|^2, 0)))  with q = outputs @ (R*s)^T + t.

Strategy:
  Host: transform queries, KD-split queries into 256 spatial groups of 16 and
  means into ~2k spatial blocks; for each group keep only blocks that could
  possibly contain a neighbor closer than sqrt(margin) (provably sound: a
  block is excluded only if every mean in it is farther than sqrt(margin)
  from every query of the group, in which case those means can neither be
  the argmin of a query whose min is below margin, nor affect the loss).
  Groups are packed 8-per-bundle into block-diagonal K=128 stationary
  operands; each candidate mean is encoded as 16 bf16 rows (hi/lo split of
  coords and squared norms) so a single bf16 matmul produces the exact-to-
  ~1e-6 squared distance in fp32 PSUM. Candidate lists are packed into
  variable-width bundle stripes (long groups split across cells, the host
  min-combines); all 8 NeuronCores run one identical NEFF on their own
  slice (data-parallel over queries). VectorE min-reduces each PSUM tile;
  the final clamp/relu/mean runs on the host.
"""

import numpy as np
import ml_dtypes

MARGIN = 0.0625
RQ = float(np.sqrt(MARGIN))  # 0.25 — candidate radius
N_CORES = 8
B_, N_, M_ = 64, 64, 100000  # input shapes (hardcoded per spec)
Q_ = B_ * N_                 # 4096 queries
GQ = 16                      # queries per group
BG = 8                       # groups per bundle (16*8 = 128 partitions)
NBUNDLES = Q_ // (GQ * BG)   # 32
BPC = NBUNDLES // N_CORES    # bundles per core = 4
MEAN_LEAF = 16               # max means per spatial block
CHUNK = 512                  # matmul free dim / PSUM bank
BF16 = ml_dtypes.bfloat16
DUMMY_D2 = 29952.0           # bf16-exact huge distance for padding columns

_prog_cache: dict = {}


# ---------------------------------------------------------------- host helpers

def _kd_leaves(pts: np.ndarray, idx: np.ndarray, leaf: int, out: list):
    """Recursive median split into spatially tight leaves of size <= leaf."""
    n = idx.shape[0]
    if n <= leaf:
        out.append(idx)
        return
    p = pts[idx]
    dim = int(np.argmax(p.max(0) - p.min(0)))
    order = np.argsort(p[:, dim], kind="stable")
    h = n // 2
    _kd_leaves(pts, idx[order[:h]], leaf, out)
    _kd_leaves(pts, idx[order[h:]], leaf, out)


def _hilo(x64: np.ndarray):
    hi = x64.astype(BF16)
    lo = (x64 - hi.astype(np.float64)).astype(BF16)
    return hi, lo


def _encode_means(means: np.ndarray) -> np.ndarray:
    """[16, M] bf16 rows: mh(3) ml(3) mh(3) ml(3) mmh mml 1 1."""
    m = means.astype(np.float64)
    mh, ml = _hilo(m)
    mm = (m * m).sum(1)
    mmh, mml = _hilo(mm)
    one = np.ones(m.shape[0], BF16)
    rows = [mh[:, 0], mh[:, 1], mh[:, 2], ml[:, 0], ml[:, 1], ml[:, 2],
            mh[:, 0], mh[:, 1], mh[:, 2], ml[:, 0], ml[:, 1], ml[:, 2],
            mmh, mml, one, one]
    return np.stack(rows).astype(BF16)


def _encode_queries(q: np.ndarray) -> np.ndarray:
    """[16, Q] bf16 rows: -2qh(3) -2qh(3) -2ql(3) -2ql(3) 1 1 qqh qql."""
    q64 = q.astype(np.float64)
    qh, ql = _hilo(q64)
    n2qh = (-2.0 * qh.astype(np.float64)).astype(BF16)  # exact scale by 2
    n2ql = (-2.0 * ql.astype(np.float64)).astype(BF16)
    qq = (q64 * q64).sum(1)
    qqh, qql = _hilo(qq)
    one = np.ones(q.shape[0], BF16)
    rows = [n2qh[:, 0], n2qh[:, 1], n2qh[:, 2], n2qh[:, 0], n2qh[:, 1], n2qh[:, 2],
            n2ql[:, 0], n2ql[:, 1], n2ql[:, 2], n2ql[:, 0], n2ql[:, 1], n2ql[:, 2],
            one, one, qqh, qql]
    return np.stack(rows).astype(BF16)


def _transform_queries(outputs, c2ws, scene_scales) -> np.ndarray:
    aff = c2ws[:, :3, :3].astype(np.float64) * scene_scales.astype(np.float64)[:, None, None]
    trans = c2ws[:, :3, 3].astype(np.float64)
    q = np.einsum("bnj,bij->bni", outputs.astype(np.float64), aff) + trans[:, None, :]
    return q.reshape(-1, 3)


def _build_work(q: np.ndarray, means: np.ndarray):
    """Partition queries/means spatially, compute certified candidate lists,
    pack into per-core bundle arrays. Returns (in_maps_data, meta)."""
    # spatial groups of queries (exactly GQ each since Q_ is a power of two)
    qgroups: list = []
    _kd_leaves(q, np.arange(Q_), GQ, qgroups)
    assert all(len(g) == GQ for g in qgroups) and len(qgroups) == Q_ // GQ

    mblocks: list = []
    _kd_leaves(means, np.arange(means.shape[0]), MEAN_LEAF, mblocks)
    centers = np.stack([means[b].mean(0) for b in mblocks])
    radii = np.array([np.sqrt(((means[b] - c) ** 2).sum(1).max())
                      for b, c in zip(mblocks, centers)])

    # include block for group iff some query is within RQ + r of its center;
    # squared-distance form via one f32 GEMM (fast, and conservative slack
    # added below absorbs the f32 rounding)
    ng, nb = len(qgroups), len(mblocks)
    qs = q[np.concatenate(qgroups)].astype(np.float32)        # [Q, 3]
    cs = centers.astype(np.float32)                           # [nb, 3]
    d2g = ((qs * qs).sum(1)[:, None] + (cs * cs).sum(1)[None, :]
           - 2.0 * (qs @ cs.T)).reshape(ng, GQ, nb).min(1)    # [ng, nb]
    thr = (RQ + radii + 1e-3).astype(np.float32) ** 2
    include = d2g <= thr[None, :]

    cand = [np.concatenate([mblocks[j] for j in np.nonzero(include[g])[0]])
            if include[g].any() else np.empty(0, np.int64)
            for g in range(ng)]
    lens = np.array([len(c) for c in cand])

    # ---- pack work into cells -------------------------------------------
    # A cell is one (bundle, lane) slot: 16 query columns x (stripe width)
    # candidate columns. Groups longer than the widest stripe are split into
    # several cells (the host min-combines duplicate query results). Empty
    # groups need no cell at all. Stripe s has 8*N_CORES = 64 cells; all 8
    # cores share one stripe width so the SPMD program is a single NEFF.
    active = [g for g in range(ng) if lens[g] > 0]
    if not active:
        return None, None, None, (), dict(ngroups=ng, nblocks=nb, total_cand=0)
    alens = lens[np.array(active)]

    best = None
    for S in range(4, 11):
        for w0 in range(1, 24):
            t = w0 * CHUNK
            npieces = int((-(-alens // t)).sum())
            if npieces > S * N_CORES * BG:
                continue
            # provisional stripe widths from sorted piece sizes
            piece_sizes = []
            for g in active:
                L = int(lens[g])
                piece_sizes += [t] * (L // t)
                if L % t:
                    piece_sizes.append(L % t)
            piece_sizes.sort(reverse=True)
            widths = []
            for s in range(S):
                chunkw = piece_sizes[s * N_CORES * BG: (s + 1) * N_CORES * BG]
                if not chunkw:
                    break
                widths.append(max(1, -(-chunkw[0] // CHUNK)))
            S_eff = len(widths)
            cost = sum(widths) + 0.8 * S_eff  # chunks + per-slot overhead
            if best is None or cost < best[0]:
                best = (cost, widths, t)
    assert best is not None
    _, widths, tsplit = best
    S = len(widths)

    # build pieces (group, start, length), sorted desc, assign to cells
    pieces = []
    for g in active:
        L = int(lens[g])
        for st in range(0, L, tsplit):
            pieces.append((g, st, min(tsplit, L - st)))
    pieces.sort(key=lambda p: -p[2])
    ncells = S * N_CORES * BG
    assert len(pieces) <= ncells

    encM = _encode_means(means)      # [16, M] bf16
    encQ = _encode_queries(q)        # [16, Q] bf16

    dummy = np.zeros((16, 1), BF16)
    dummy[12] = BF16(DUMMY_D2)       # mmh row -> huge d2
    dummy[14] = BF16(1.0)
    dummy[15] = BF16(1.0)

    XBs = []                         # per stripe: [N_CORES, 128, w*CHUNK]
    for s in range(S):
        ncc = widths[s] * CHUNK
        x = np.tile(dummy, (N_CORES, BG, ncc)).reshape(N_CORES, 128, ncc)
        XBs.append(x)
    WB = np.zeros((N_CORES, S, 128, 128), BF16)
    qmap = np.full((N_CORES, S, 128), -1, np.int64)  # psum partition -> query

    for i, (g, st, ln) in enumerate(pieces):
        s, rem = divmod(i, N_CORES * BG)
        c, k = divmod(rem, BG)
        r0 = k * GQ
        ci = cand[g][st:st + ln]
        XBs[s][c, r0:r0 + GQ, :ln] = encM[:, ci]
        WB[c, s, r0:r0 + GQ, r0:r0 + GQ] = encQ[:, qgroups[g]]
        qmap[c, s, r0:r0 + GQ] = qgroups[g]

    stats = dict(ngroups=ng, nblocks=nb, total_cand=int(lens.sum()),
                 max_len=int(lens.max()), widths=tuple(widths),
                 npieces=len(pieces), mean_len=float(lens.mean()))
    return XBs, WB, qmap, tuple(widths), stats


# ---------------------------------------------------------------- device program

def _get_program(widths: tuple):
    key = tuple(widths)
    if key in _prog_cache:
        return _prog_cache[key]

    import concourse.mybir as mybir
    import concourse.tile as tile
    import concourse.bacc as bacc

    S = len(widths)
    nc = bacc.Bacc("TRN2", target_bir_lowering=False, debug=False,
                   num_devices=N_CORES)
    xbs = [nc.declare_dram_parameter(f"xb{s}", [128, widths[s] * CHUNK],
                                     mybir.dt.bfloat16, isOutput=False)
           for s in range(S)]
    wb = nc.declare_dram_parameter("wb", [S, 128, 128], mybir.dt.bfloat16,
                                   isOutput=False)
    out = nc.declare_dram_parameter("out", [128, S], mybir.dt.float32,
                                    isOutput=True)

    with tile.TileContext(nc) as tc:
        with (
            tc.tile_pool(name="xpool", bufs=S) as xpool,
            tc.tile_pool(name="wpool", bufs=S) as wpool,
            tc.tile_pool(name="apool", bufs=2) as apool,
            tc.tile_pool(name="opool", bufs=1) as opool,
            tc.tile_pool(name="psum", bufs=8, space="PSUM") as psum,
        ):
            omerge = opool.tile([128, S], mybir.dt.float32, name="omerge")
            # prefetch every stripe's X and W upfront; DMA overlaps compute
            xts, wts = [], []
            for s in range(S):
                xt = xpool.tile([128, widths[s] * CHUNK], mybir.dt.bfloat16,
                                name=f"x{s}", tag="x")
                nc.sync.dma_start(xt[:], xbs[s][:])
                xts.append(xt)
                wt = wpool.tile([128, 128], mybir.dt.bfloat16,
                                name=f"w{s}", tag="w")
                nc.sync.dma_start(wt[:], wb[s])
                wts.append(wt)
            for s in range(S):
                acc = apool.tile([128, widths[s]], mybir.dt.float32,
                                 name=f"a{s}", tag="acc")
                for j in range(widths[s]):
                    ps = psum.tile([128, CHUNK], mybir.dt.float32,
                                   name="ps", tag="ps")
                    nc.tensor.matmul(ps[:], wts[s][:],
                                     xts[s][:, j * CHUNK:(j + 1) * CHUNK],
                                     start=True, stop=True)
                    nc.vector.tensor_reduce(acc[:, j:j + 1], ps[:],
                                            axis=mybir.AxisListType.X,
                                            op=mybir.AluOpType.min)
                nc.vector.tensor_reduce(omerge[:, s:s + 1], acc[:],
                                        axis=mybir.AxisListType.X,
                                        op=mybir.AluOpType.min)
            nc.sync.dma_start(out[:], omerge[:])
    nc.compile()
    _prog_cache[key] = nc
    return nc


# ---------------------------------------------------------------- entry points

def _finish(d2: np.ndarray) -> np.ndarray:
    dists = np.maximum(d2.astype(np.float64), 0.0)
    loss = np.maximum(MARGIN - dists, 0.0).mean()
    return np.array(loss, dtype=np.float32)


def _numpy_fallback(q: np.ndarray, means: np.ndarray) -> np.ndarray:
    m = means.astype(np.float64)
    mm = (m * m).sum(1)
    d2 = np.empty(q.shape[0])
    for i in range(0, q.shape[0], 256):
        qc = q[i:i + 256]
        d = (qc * qc).sum(1)[:, None] + mm[None, :] - 2.0 * (qc @ m.T)
        d2[i:i + 256] = d.min(1)
    return _finish(d2)


def kernel(outputs, c2ws, scene_scales, means):
    outputs = np.asarray(outputs)
    c2ws = np.asarray(c2ws)
    scene_scales = np.asarray(scene_scales)
    means = np.asarray(means)

    q = _transform_queries(outputs, c2ws, scene_scales)
    try:
        XBs, WB, qmap, widths, stats = _build_work(q, means)
    except Exception:
        return _numpy_fallback(q, means)
    kernel.last_stats = stats

    if len(widths) == 0:  # no query anywhere near a mean: every term is 0
        return _finish(np.full(Q_, np.inf))
    if sum(widths) > 400:  # pathological input: pruning failed; do it on host
        return _numpy_fallback(q, means)

    nc = _get_program(widths)
    from concourse.bass_utils import run_bass_kernel_spmd

    S = len(widths)
    in_maps = [dict(**{f"xb{s}": np.ascontiguousarray(XBs[s][c])
                       for s in range(S)},
                    wb=np.ascontiguousarray(WB[c]))
               for c in range(N_CORES)]
    res = run_bass_kernel_spmd(nc, in_maps, list(range(N_CORES)))
    kernel.last_run = (nc, in_maps)  # for external profiling/timing harnesses

    # split groups appear in several cells; combine with elementwise min.
    # queries never assigned to a cell keep +inf -> loss term 0.
    d2 = np.full(Q_, np.inf, np.float64)
    for c in range(N_CORES):
        o = np.asarray(res.results[c]["out"], np.float64).T.ravel()  # [S,128]
        qm = qmap[c].ravel()
        valid = qm >= 0
        np.minimum.at(d2, qm[valid], o[valid])
    return _finish(d2)


# revision 20
# speedup vs baseline: 1.0317x; 1.0317x over previous
"""Trainium2 kernel for nn_NNLoss (brute-force NN + margin loss).

loss = mean(relu(margin - max(min_j |q_i - m_j|^2, 0)))  with q = outputs @ (R*s)^T + t.

Strategy:
  Host: transform queries, KD-split queries into 256 spatial groups of 16 and
  means into ~2k spatial blocks; for each group keep only blocks that could
  possibly contain a neighbor closer than sqrt(margin) (provably sound: a
  block is excluded only if every mean in it is farther than sqrt(margin)
  from every query of the group, in which case those means can neither be
  the argmin of a query whose min is below margin, nor affect the loss).
  Groups are packed 8-per-bundle into block-diagonal K=128 stationary
  operands; each candidate mean is encoded as 16 bf16 rows (hi/lo split of
  coords and squared norms) so a single bf16 matmul produces the exact-to-
  ~1e-6 squared distance in fp32 PSUM. Candidate lists are packed into
  variable-width bundle stripes (long groups split across cells, the host
  min-combines); all 8 NeuronCores run one identical NEFF on their own
  slice (data-parallel over queries). VectorE min-reduces each PSUM tile;
  the final clamp/relu/mean runs on the host.
"""

import numpy as np
import ml_dtypes

MARGIN = 0.0625
RQ = float(np.sqrt(MARGIN))  # 0.25 — candidate radius
N_CORES = 8
B_, N_, M_ = 64, 64, 100000  # input shapes (hardcoded per spec)
Q_ = B_ * N_                 # 4096 queries
GQ = 16                      # queries per group
BG = 8                       # groups per bundle (16*8 = 128 partitions)
NBUNDLES = Q_ // (GQ * BG)   # 32
BPC = NBUNDLES // N_CORES    # bundles per core = 4
MEAN_LEAF = 16               # max means per spatial block
CHUNK = 512                  # matmul free dim / PSUM bank
BF16 = ml_dtypes.bfloat16
DUMMY_D2 = 29952.0           # bf16-exact huge distance for padding columns

_prog_cache: dict = {}


# ---------------------------------------------------------------- host helpers

def _kd_leaves(pts: np.ndarray, idx: np.ndarray, leaf: int, out: list):
    """Recursive median split into spatially tight leaves of size <= leaf."""
    n = idx.shape[0]
    if n <= leaf:
        out.append(idx)
        return
    p = pts[idx]
    dim = int(np.argmax(p.max(0) - p.min(0)))
    order = np.argsort(p[:, dim], kind="stable")
    h = n // 2
    _kd_leaves(pts, idx[order[:h]], leaf, out)
    _kd_leaves(pts, idx[order[h:]], leaf, out)


def _hilo(x64: np.ndarray):
    hi = x64.astype(BF16)
    lo = (x64 - hi.astype(np.float64)).astype(BF16)
    return hi, lo


def _encode_means(means: np.ndarray) -> np.ndarray:
    """[16, M] bf16 rows: mh(3) ml(3) mh(3) ml(3) mmh mml 1 1."""
    m = means.astype(np.float64)
    mh, ml = _hilo(m)
    mm = (m * m).sum(1)
    mmh, mml = _hilo(mm)
    one = np.ones(m.shape[0], BF16)
    rows = [mh[:, 0], mh[:, 1], mh[:, 2], ml[:, 0], ml[:, 1], ml[:, 2],
            mh[:, 0], mh[:, 1], mh[:, 2], ml[:, 0], ml[:, 1], ml[:, 2],
            mmh, mml, one, one]
    return np.stack(rows).astype(BF16)


def _encode_queries(q: np.ndarray) -> np.ndarray:
    """[16, Q] bf16 rows: -2qh(3) -2qh(3) -2ql(3) -2ql(3) 1 1 qqh qql."""
    q64 = q.astype(np.float64)
    qh, ql = _hilo(q64)
    n2qh = (-2.0 * qh.astype(np.float64)).astype(BF16)  # exact scale by 2
    n2ql = (-2.0 * ql.astype(np.float64)).astype(BF16)
    qq = (q64 * q64).sum(1)
    qqh, qql = _hilo(qq)
    one = np.ones(q.shape[0], BF16)
    rows = [n2qh[:, 0], n2qh[:, 1], n2qh[:, 2], n2qh[:, 0], n2qh[:, 1], n2qh[:, 2],
            n2ql[:, 0], n2ql[:, 1], n2ql[:, 2], n2ql[:, 0], n2ql[:, 1], n2ql[:, 2],
            one, one, qqh, qql]
    return np.stack(rows).astype(BF16)


def _transform_queries(outputs, c2ws, scene_scales) -> np.ndarray:
    aff = c2ws[:, :3, :3].astype(np.float64) * scene_scales.astype(np.float64)[:, None, None]
    trans = c2ws[:, :3, 3].astype(np.float64)
    q = np.einsum("bnj,bij->bni", outputs.astype(np.float64), aff) + trans[:, None, :]
    return q.reshape(-1, 3)


def _build_work(q: np.ndarray, means: np.ndarray):
    """Partition queries/means spatially, compute certified candidate lists,
    pack into per-core bundle arrays. Returns (in_maps_data, meta)."""
    # spatial groups of queries (exactly GQ each since Q_ is a power of two)
    qgroups: list = []
    _kd_leaves(q, np.arange(Q_), GQ, qgroups)
    assert all(len(g) == GQ for g in qgroups) and len(qgroups) == Q_ // GQ

    mblocks: list = []
    _kd_leaves(means, np.arange(means.shape[0]), MEAN_LEAF, mblocks)
    centers = np.stack([means[b].mean(0) for b in mblocks])
    radii = np.array([np.sqrt(((means[b] - c) ** 2).sum(1).max())
                      for b, c in zip(mblocks, centers)])

    # include block for group iff some query is within RQ + r of its center;
    # squared-distance form via one f32 GEMM (fast, and conservative slack
    # added below absorbs the f32 rounding)
    ng, nb = len(qgroups), len(mblocks)
    qs = q[np.concatenate(qgroups)].astype(np.float32)        # [Q, 3]
    cs = centers.astype(np.float32)                           # [nb, 3]
    d2g = ((qs * qs).sum(1)[:, None] + (cs * cs).sum(1)[None, :]
           - 2.0 * (qs @ cs.T)).reshape(ng, GQ, nb).min(1)    # [ng, nb]
    thr = (RQ + radii + 1e-3).astype(np.float32) ** 2
    include = d2g <= thr[None, :]

    cand = [np.concatenate([mblocks[j] for j in np.nonzero(include[g])[0]])
            if include[g].any() else np.empty(0, np.int64)
            for g in range(ng)]
    lens = np.array([len(c) for c in cand])

    # ---- pack work into cells -------------------------------------------
    # A cell is one (bundle, lane) slot: 16 query columns x (stripe width)
    # candidate columns. Groups longer than the widest stripe are split into
    # several cells (the host min-combines duplicate query results). Empty
    # groups need no cell at all. Stripe s has 8*N_CORES = 64 cells; all 8
    # cores share one stripe width so the SPMD program is a single NEFF.
    active = [g for g in range(ng) if lens[g] > 0]
    if not active:
        return None, None, None, (), dict(ngroups=ng, nblocks=nb, total_cand=0)
    alens = lens[np.array(active)]

    best = None
    for S in range(4, 11):
        for w0 in range(1, 24):
            t = w0 * CHUNK
            npieces = int((-(-alens // t)).sum())
            if npieces > S * N_CORES * BG:
                continue
            # provisional stripe widths from sorted piece sizes
            piece_sizes = []
            for g in active:
                L = int(lens[g])
                piece_sizes += [t] * (L // t)
                if L % t:
                    piece_sizes.append(L % t)
            piece_sizes.sort(reverse=True)
            widths = []
            for s in range(S):
                chunkw = piece_sizes[s * N_CORES * BG: (s + 1) * N_CORES * BG]
                if not chunkw:
                    break
                widths.append(max(1, -(-chunkw[0] // CHUNK)))
            S_eff = len(widths)
            cost = sum(widths) + 0.8 * S_eff  # chunks + per-slot overhead
            if best is None or cost < best[0]:
                best = (cost, widths, t)
    assert best is not None
    _, widths, tsplit = best
    S = len(widths)

    # build pieces (group, start, length), sorted desc, assign to cells
    pieces = []
    for g in active:
        L = int(lens[g])
        for st in range(0, L, tsplit):
            pieces.append((g, st, min(tsplit, L - st)))
    pieces.sort(key=lambda p: -p[2])
    ncells = S * N_CORES * BG
    assert len(pieces) <= ncells

    encM = _encode_means(means)      # [16, M] bf16
    encQ = _encode_queries(q)        # [16, Q] bf16

    dummy = np.zeros((16, 1), BF16)
    dummy[12] = BF16(DUMMY_D2)       # mmh row -> huge d2
    dummy[14] = BF16(1.0)
    dummy[15] = BF16(1.0)

    XBs = []                         # per stripe: [N_CORES, 128, w*CHUNK]
    for s in range(S):
        ncc = widths[s] * CHUNK
        x = np.tile(dummy, (N_CORES, BG, ncc)).reshape(N_CORES, 128, ncc)
        XBs.append(x)
    WB = np.zeros((N_CORES, 128, S * 128), BF16)  # all slots' W side by side
    qmap = np.full((N_CORES, S, 128), -1, np.int64)  # psum partition -> query

    for i, (g, st, ln) in enumerate(pieces):
        s, rem = divmod(i, N_CORES * BG)
        c, k = divmod(rem, BG)
        r0 = k * GQ
        ci = cand[g][st:st + ln]
        XBs[s][c, r0:r0 + GQ, :ln] = encM[:, ci]
        WB[c, r0:r0 + GQ, s * 128 + r0:s * 128 + r0 + GQ] = encQ[:, qgroups[g]]
        qmap[c, s, r0:r0 + GQ] = qgroups[g]

    stats = dict(ngroups=ng, nblocks=nb, total_cand=int(lens.sum()),
                 max_len=int(lens.max()), widths=tuple(widths),
                 npieces=len(pieces), mean_len=float(lens.mean()))
    return XBs, WB, qmap, tuple(widths), stats


# ---------------------------------------------------------------- device program

def _get_program(widths: tuple):
    key = tuple(widths)
    if key in _prog_cache:
        return _prog_cache[key]

    import concourse.mybir as mybir
    import concourse.tile as tile
    import concourse.bacc as bacc

    S = len(widths)
    nc = bacc.Bacc("TRN2", target_bir_lowering=False, debug=False,
                   num_devices=N_CORES)
    xbs = [nc.declare_dram_parameter(f"xb{s}", [128, widths[s] * CHUNK],
                                     mybir.dt.bfloat16, isOutput=False)
           for s in range(S)]
    wb = nc.declare_dram_parameter("wb", [128, S * 128], mybir.dt.bfloat16,
                                   isOutput=False)
    out = nc.declare_dram_parameter("out", [128, S], mybir.dt.float32,
                                    isOutput=True)

    with tile.TileContext(nc) as tc:
        with (
            tc.tile_pool(name="xpool", bufs=S) as xpool,
            tc.tile_pool(name="wpool", bufs=1) as wpool,
            tc.tile_pool(name="apool", bufs=2) as apool,
            tc.tile_pool(name="opool", bufs=1) as opool,
            tc.tile_pool(name="psum", bufs=8, space="PSUM") as psum,
        ):
            omerge = opool.tile([128, S], mybir.dt.float32, name="omerge")
            # prefetch all stationary operands (one DMA) and every stripe's X
            wt = wpool.tile([128, S * 128], mybir.dt.bfloat16, name="wall")
            nc.sync.dma_start(wt[:], wb[:])
            xts = []
            for s in range(S):
                xt = xpool.tile([128, widths[s] * CHUNK], mybir.dt.bfloat16,
                                name=f"x{s}", tag="x")
                nc.sync.dma_start(xt[:], xbs[s][:])
                xts.append(xt)
            for s in range(S):
                w = widths[s]
                ws = wt[:, s * 128:(s + 1) * 128]
                if w == 1:  # single chunk: reduce PSUM straight into omerge
                    ps = psum.tile([128, CHUNK], mybir.dt.float32,
                                   name="ps", tag="ps")
                    nc.tensor.matmul(ps[:], ws, xts[s][:], start=True, stop=True)
                    nc.vector.tensor_reduce(omerge[:, s:s + 1], ps[:],
                                            axis=mybir.AxisListType.X,
                                            op=mybir.AluOpType.min)
                    continue
                acc = apool.tile([128, w], mybir.dt.float32,
                                 name=f"a{s}", tag="acc")
                for j in range(w):
                    ps = psum.tile([128, CHUNK], mybir.dt.float32,
                                   name="ps", tag="ps")
                    nc.tensor.matmul(ps[:], ws,
                                     xts[s][:, j * CHUNK:(j + 1) * CHUNK],
                                     start=True, stop=True)
                    nc.vector.tensor_reduce(acc[:, j:j + 1], ps[:],
                                            axis=mybir.AxisListType.X,
                                            op=mybir.AluOpType.min)
                nc.vector.tensor_reduce(omerge[:, s:s + 1], acc[:],
                                        axis=mybir.AxisListType.X,
                                        op=mybir.AluOpType.min)
            nc.sync.dma_start(out[:], omerge[:])
    nc.compile()
    _prog_cache[key] = nc
    return nc


# ---------------------------------------------------------------- entry points

def _finish(d2: np.ndarray) -> np.ndarray:
    dists = np.maximum(d2.astype(np.float64), 0.0)
    loss = np.maximum(MARGIN - dists, 0.0).mean()
    return np.array(loss, dtype=np.float32)


def _numpy_fallback(q: np.ndarray, means: np.ndarray) -> np.ndarray:
    m = means.astype(np.float64)
    mm = (m * m).sum(1)
    d2 = np.empty(q.shape[0])
    for i in range(0, q.shape[0], 256):
        qc = q[i:i + 256]
        d = (qc * qc).sum(1)[:, None] + mm[None, :] - 2.0 * (qc @ m.T)
        d2[i:i + 256] = d.min(1)
    return _finish(d2)


def kernel(outputs, c2ws, scene_scales, means):
    outputs = np.asarray(outputs)
    c2ws = np.asarray(c2ws)
    scene_scales = np.asarray(scene_scales)
    means = np.asarray(means)

    q = _transform_queries(outputs, c2ws, scene_scales)
    try:
        XBs, WB, qmap, widths, stats = _build_work(q, means)
    except Exception:
        return _numpy_fallback(q, means)
    kernel.last_stats = stats

    if len(widths) == 0:  # no query anywhere near a mean: every term is 0
        return _finish(np.full(Q_, np.inf))
    if sum(widths) > 400:  # pathological input: pruning failed; do it on host
        return _numpy_fallback(q, means)

    nc = _get_program(widths)
    from concourse.bass_utils import run_bass_kernel_spmd

    S = len(widths)
    in_maps = [dict(**{f"xb{s}": np.ascontiguousarray(XBs[s][c])
                       for s in range(S)},
                    wb=np.ascontiguousarray(WB[c]))
               for c in range(N_CORES)]
    res = run_bass_kernel_spmd(nc, in_maps, list(range(N_CORES)))
    kernel.last_run = (nc, in_maps)  # for external profiling/timing harnesses

    # split groups appear in several cells; combine with elementwise min.
    # queries never assigned to a cell keep +inf -> loss term 0.
    d2 = np.full(Q_, np.inf, np.float64)
    for c in range(N_CORES):
        o = np.asarray(res.results[c]["out"], np.float64).T.ravel()  # [S,128]
        qm = qmap[c].ravel()
        valid = qm >= 0
        np.minimum.at(d2, qm[valid], o[valid])
    return _finish(d2)
